# revision 12
# baseline (speedup 1.0000x reference)
"""DeepSeek-MLA attention kernel for 8 Trainium2 NeuronCores.

Sharding: tensor-parallel over heads (4 of 16 per core) x data-parallel over
batch (1 of 2 per core-group).  Core c handles batch c//4, heads
[4*(c%4), 4*(c%4)+4).  Each core computes a partial [HID, TOK] output (its
heads' contribution through Wo); the host sums the 4 partials per batch and
transposes back.

Projection paths run in bf16 (PE cost identical to fp32r, half the DMA/SBUF);
attention scores accumulate in f32 PSUM.  Softmax sums ride the Pool engine
(tensor_add tree + partition_all_reduce) instead of PE ones-matmuls, and the
causal mask is a 0/1 multiply on the exp output (Pool) rather than f32 adds
on PSUM.  Weights are loaded once; hidden states stream in token quarters
with fully-contiguous DMA.  RoPE uses a pair-interleaved row permutation so
rotate_half is a 32-lane stream_shuffle on the vector engine.
"""

import math
import sys

import numpy as np

for _p in ("/opt/trn_rl_repo", "/root/.axon_site/_ro/trn_rl_repo"):
    if _p not in sys.path:
        sys.path.append(_p)

# Problem dims (hardcoded per contract)
B, S, HID = 2, 2048, 2048
H, DN, DR, DV, R = 16, 128, 64, 128, 512
QHD = DN + DR  # 192
EPS = 1e-5
N_CORES = 8
NH = 4                 # heads per core
TOK = S                # tokens per core (one batch)
QROWS = NH * QHD       # 768 = 4*128 nope + 2*128 packed rope
WCOLS = QROWS + R      # 1280 fused wq|wd columns
NEG = -30000.0

_CACHE = {}

_SHUF_MASK = []
for _i in range(16):
    _SHUF_MASK += [2 * _i + 1, 2 * _i]


# ----------------------------------------------------------------------------
# Device program
# ----------------------------------------------------------------------------

def _build(causal: bool):
    import concourse.mybir as mybir
    import concourse.tile as tile
    from concourse import bacc

    F32 = mybir.dt.float32
    BF16 = mybir.dt.bfloat16

    nc = bacc.Bacc("TRN2", target_bir_lowering=False, debug=False,
                   enable_asserts=False, num_devices=N_CORES)

    # hT stored token-quarter-major: rows 2048*tq + 128*c .. +128 hold HID
    # chunk c for token quarter tq; each [128, 512] chunk is contiguous.
    hTd = nc.dram_tensor("hTd", [4 * HID, 512], BF16, kind="ExternalInput").ap()
    wqdd = nc.dram_tensor("wqdd", [HID, WCOLS], BF16, kind="ExternalInput").ap()
    wud = nc.dram_tensor("wud", [R, QROWS], BF16, kind="ExternalInput").ap()
    wvd = nc.dram_tensor("wvd", [R, NH * DV], BF16, kind="ExternalInput").ap()
    wod = nc.dram_tensor("wod", [NH * DV, HID], BF16, kind="ExternalInput").ap()
    cosd = nc.dram_tensor("cosd", [128, TOK], BF16, kind="ExternalInput").ap()
    sind = nc.dram_tensor("sind", [128, TOK], BF16, kind="ExternalInput").ap()
    if causal:
        dmd = nc.dram_tensor("dmd", [128, 4 * 512], BF16, kind="ExternalInput").ap()
    else:
        maskTd = nc.dram_tensor("maskTd", [S, S], F32, kind="ExternalInput").ap()
    outd = nc.dram_tensor("out", [HID, TOK], F32, kind="ExternalOutput").ap()

    with tile.TileContext(nc) as tc:
        with nc.allow_low_precision(reason="bf16/f32r mixed-precision kernel"):
            _emit(tc, nc, mybir, causal,
                  hTd, wqdd, wud, wvd, wod, cosd, sind,
                  dmd if causal else maskTd, outd)
    nc.compile()
    return nc


def _emit(tc, nc, mybir, causal,
          hTd, wqdd, wud, wvd, wod, cosd, sind, maskd, outd):
    from concourse import bass_isa

    F32 = mybir.dt.float32
    BF16 = mybir.dt.bfloat16
    Exp = mybir.ActivationFunctionType.Exp
    Sqrt = mybir.ActivationFunctionType.Sqrt
    ADD = bass_isa.ReduceOp.add

    # --- pool A: tiles visible during P1 that live past it ---
    pA = tc.alloc_tile_pool(name="A", bufs=1)
    dm01 = pA.tile([128, 4 * 512], BF16, name="dm01", tag="dm01") if causal else None
    eps_t = pA.tile([128, 1], F32, name="eps_t", tag="eps_t")
    nc.vector.memset(eps_t[:], EPS)
    qn = [pA.tile([128, TOK], BF16, name=f"qn{h}", tag=f"qn{h}") for h in range(4)]
    qr = [pA.tile([128, TOK], BF16, name=f"qr{p}", tag=f"qr{p}") for p in range(2)]
    cos_t = pA.tile([128, TOK], BF16, name="cos_t", tag="cos_t")
    sin_t = pA.tile([128, TOK], BF16, name="sin_t", tag="sin_t")
    cv = [pA.tile([128, TOK], BF16, name=f"cv{i}", tag=f"cv{i}") for i in range(4)]
    wu_ch = [pA.tile([128, QROWS], BF16, name=f"wu{i}", tag=f"wu{i}")
             for i in range(4)]
    wv_ch = [pA.tile([128, NH * DV], BF16, name=f"wv{i}", tag=f"wv{i}")
             for i in range(4)]
    kr_raw = [pA.tile([128, TOK], BF16, name=f"kr_raw{p}", tag=f"kr_raw{p}")
              for p in range(2)]
    qr_raw = [pA.tile([128, TOK], BF16, name=f"qr_raw{p}", tag=f"qr_raw{p}")
              for p in range(2)]

    # ------------------------------------------------------------------
    # P1: q / c_kv projections, streamed in token quarters.
    #     M-tile columns in wqd: 0-511 qn(4), 512-767 qr(2), 768-1279 cv(4)
    # ------------------------------------------------------------------
    pP1 = tc.alloc_tile_pool(name="P1", bufs=1)
    wqd_ch = [pP1.tile([128, WCOLS], BF16, name=f"wqd{c}", tag=f"wqd{c}")
              for c in range(16)]

    # m-tile order: cv first (so RMSNorm can overlap), then qn, then qr
    m_tiles = ([(768 + 128 * i, cv[i]) for i in range(4)]
               + [(128 * h, qn[h]) for h in range(4)]
               + [(512 + 128 * p, qr_raw[p]) for p in range(2)])

    with tc.tile_pool(name="ht", bufs=20) as ph, \
         tc.tile_pool(name="p1ps", bufs=4, space="PSUM") as pps:
        # critical-path loads first: wqd chunk + tq0 hT chunk per c, both
        # queues fed; aux loads (cos/sin/mask/wu/wv) follow.
        ht0 = []
        for c in range(16):
            e1, e2 = (nc.sync, nc.scalar) if c % 2 == 0 else (nc.scalar, nc.sync)
            e1.dma_start(wqd_ch[c][:], wqdd[128 * c:128 * (c + 1), :])
            t = ph.tile([128, 512], BF16, name=f"ht0_{c}", tag="ht")
            e2.dma_start(t[:], hTd[128 * c:128 * (c + 1), :])
            ht0.append(t)
        nc.sync.dma_start(cos_t[:], cosd)
        nc.sync.dma_start(sin_t[:], sind)
        if causal:
            nc.sync.dma_start(dm01[:], maskd)
        for i in range(4):
            nc.sync.dma_start(wu_ch[i][:], wud[128 * i:128 * (i + 1), :])
            nc.sync.dma_start(wv_ch[i][:], wvd[128 * i:128 * (i + 1), :])

        for tq in range(4):
            if tq == 0:
                ht = ht0
            else:
                ht = []
                for c in range(16):
                    t = ph.tile([128, 512], BF16, name=f"ht{tq}_{c}", tag="ht")
                    eng = nc.sync if c % 2 == 0 else nc.scalar
                    eng.dma_start(t[:], hTd[2048 * tq + 128 * c:
                                            2048 * tq + 128 * (c + 1), :])
                    ht.append(t)
            for mi, (mcol, target) in enumerate(m_tiles):
                ps = pps.tile([128, 512], F32, name=f"p1ps{tq}_{mi}", tag="p1ps")
                for c in range(16):
                    nc.tensor.matmul(ps[:], wqd_ch[c][:, mcol:mcol + 128],
                                     ht[c][:], start=(c == 0), stop=(c == 15))
                nc.vector.tensor_copy(target[:, 512 * tq:512 * (tq + 1)], ps[:])
                if mi == 3:
                    # P2: RMSNorm for this quarter (Pool/Act/DVE, no PE) —
                    # emitted early so its serial tail hides under qn/qr passes
                    _rmsnorm_tq(tc, nc, mybir, ADD, cv, eps_t, tq)
    pP1.release()

    # --- pool B: P3 outputs + output-projection tiles (fit where P1 was) ---
    pB = tc.alloc_tile_pool(name="Bpool", bufs=1)
    kn = [pB.tile([128, TOK], BF16, name=f"kn{h}", tag=f"kn{h}") for h in range(4)]
    kr = [pB.tile([128, TOK], BF16, name=f"kr{p}", tag=f"kr{p}") for p in range(2)]
    v_sb = [pB.tile([128, NH * DV], BF16, name=f"v{t}", tag=f"v{t}")
            for t in range(16)]
    o_sb = [[pB.tile([128, 512], BF16, name=f"o{h}_{j}", tag=f"o{h}_{j}")
             for j in range(4)] for h in range(4)]
    wo_ch = [pB.tile([128, HID], BF16, name=f"wo{i}", tag=f"wo{i}")
             for i in range(4)]

    # ------------------------------------------------------------------
    # P3a: k up-projection (+ k RoPE);  P3b: v up-projection
    # ------------------------------------------------------------------
    k_out = [(kr_raw[0], 512), (kr_raw[1], 640),
             (kn[0], 0), (kn[1], 128), (kn[2], 256), (kn[3], 384)]
    with tc.tile_pool(name="p3ps", bufs=4, space="PSUM") as ppk:
        for mi, (target, coff) in enumerate(k_out):
            for s in range(4):
                ps = ppk.tile([128, 512], F32, name=f"p3k{mi}_{s}", tag="p3k")
                for i in range(4):
                    nc.tensor.matmul(ps[:], wu_ch[i][:, coff:coff + 128],
                                     cv[i][:, 512 * s:512 * (s + 1)],
                                     start=(i == 0), stop=(i == 3))
                if (mi * 4 + s) % 2 == 0:
                    nc.scalar.activation(target[:, 512 * s:512 * (s + 1)], ps[:],
                                         mybir.ActivationFunctionType.Copy)
                else:
                    nc.vector.tensor_copy(target[:, 512 * s:512 * (s + 1)], ps[:])
    with tc.tile_pool(name="krope", bufs=2) as pkt:
        for p in range(2):
            tmp = pkt.tile([128, TOK], BF16, name=f"kropetmp{p}", tag="kropetmp")
            nc.vector.stream_shuffle(tmp[:], kr_raw[p][:], _SHUF_MASK)
            nc.vector.tensor_mul(tmp[:], tmp[:], sin_t[:])
            nc.vector.tensor_mul(kr_raw[p][:], kr_raw[p][:], cos_t[:])
            nc.vector.tensor_add(kr[p][:], kr_raw[p][:], tmp[:])
    with tc.tile_pool(name="p3vps", bufs=4, space="PSUM") as ppv:
        for tt in range(16):
            ps = ppv.tile([128, NH * DV], F32, name=f"p3v{tt}", tag="p3v")
            for i in range(4):
                nc.tensor.matmul(ps[:], cv[i][:, 128 * tt:128 * (tt + 1)],
                                 wv_ch[i][:], start=(i == 0), stop=(i == 3))
            if tt % 2 == 0:
                nc.scalar.activation(v_sb[tt][:], ps[:],
                                     mybir.ActivationFunctionType.Copy)
            else:
                nc.vector.tensor_copy(v_sb[tt][:], ps[:])

    # RoPE on q (DVE; overlaps P3b PE work)
    with tc.tile_pool(name="qrope", bufs=2) as pr:
        for p in range(2):
            tmp = pr.tile([128, TOK], BF16, name=f"qropetmp{p}", tag="qropetmp")
            nc.vector.stream_shuffle(tmp[:], qr_raw[p][:], _SHUF_MASK)
            nc.vector.tensor_mul(tmp[:], tmp[:], sin_t[:])
            nc.vector.tensor_mul(qr_raw[p][:], qr_raw[p][:], cos_t[:])
            nc.vector.tensor_add(qr[p][:], qr_raw[p][:], tmp[:])

    # wo loads (Act queue is about to go exp-only; issue on SP)
    for i in range(4):
        nc.sync.dma_start(wo_ch[i][:], wod[128 * i:128 * (i + 1), :])

    # ------------------------------------------------------------------
    # P4: attention per head; transposed scores sT[k, q].
    #     Sum of exp rides Pool (add tree + partition_all_reduce).
    # ------------------------------------------------------------------
    pm = None if causal else tc.alloc_tile_pool(name="mload", bufs=4)
    with tc.tile_pool(name="exp", bufs=6) as pe_, \
         tc.tile_pool(name="acc", bufs=2) as pa_, \
         tc.tile_pool(name="norm", bufs=4) as pn, \
         tc.tile_pool(name="fout", bufs=4) as pf, \
         tc.tile_pool(name="qkps", bufs=3, space="PSUM") as pqk, \
         tc.tile_pool(name="pvps", bufs=3, space="PSUM") as ppv4, \
         tc.tile_pool(name="p5ps", bufs=2, space="PSUM") as pps5:
        for j in range(4):
            nch = 4 * (j + 1) if causal else 16
            for h in range(4):
                p = h // 2
                rs0 = 64 * (h % 2)
                pv_ps = ppv4.tile([128, 512], F32, name=f"pv{h}_{j}", tag="pv")
                acc = pa_.tile([128, 512], F32, name=f"acc{h}_{j}", tag="acc")
                for ci in range(nch):
                    c = ci
                    qk_ps = pqk.tile([128, 512], F32, name=f"qk{h}_{j}_{c}", tag="qk")
                    nc.tensor.matmul(qk_ps[:],
                                     kn[h][:, 128 * c:128 * (c + 1)],
                                     qn[h][:, 512 * j:512 * (j + 1)],
                                     start=True, stop=False)
                    nc.tensor.matmul(qk_ps[:],
                                     kr[p][rs0:rs0 + 64, 128 * c:128 * (c + 1)],
                                     qr[p][rs0:rs0 + 64, 512 * j:512 * (j + 1)],
                                     start=False, stop=True)
                    if not causal:
                        mt = pm.tile([128, 512], F32, name=f"mt{h}{j}{c}", tag="mt")
                        eng = nc.sync if ci % 2 == 0 else nc.scalar
                        eng.dma_start(mt[:], maskd[128 * c:128 * (c + 1),
                                                   512 * j:512 * (j + 1)])
                        nc.vector.tensor_add(qk_ps[:], qk_ps[:], mt[:])
                    e = pe_.tile([128, 512], BF16, name=f"e{h}{j}{c}", tag="e")
                    nc.scalar.activation(e[:], qk_ps[:], Exp)
                    if causal:
                        d = c - 4 * j
                        if d >= 0:
                            nc.gpsimd.tensor_mul(e[:], e[:],
                                                 dm01[:, 512 * d:512 * (d + 1)])
                    if ci == 0:
                        nc.gpsimd.tensor_copy(acc[:], e[:])
                    else:
                        nc.gpsimd.tensor_add(acc[:], acc[:], e[:])
                    nc.tensor.matmul(pv_ps[:],
                                     v_sb[c][:, 128 * h:128 * (h + 1)],
                                     e[:],
                                     start=(ci == 0), stop=(ci == nch - 1))
                nc.gpsimd.partition_all_reduce(acc[:], acc[:], 128, ADD)
                rr = pn.tile([128, 512], F32, name=f"rr{h}{j}", tag="rr")
                nc.vector.reciprocal(rr[:], acc[:])
                nc.vector.tensor_mul(o_sb[h][j][:], pv_ps[:], rr[:])
            # P5 for this q block: fills exp-wait bubbles, shrinks the drain
            for dt in range(16):
                ps = pps5.tile([128, 512], F32, name=f"p5_{j}_{dt}", tag="p5")
                for i in range(4):
                    nc.tensor.matmul(ps[:],
                                     wo_ch[i][:, 128 * dt:128 * (dt + 1)],
                                     o_sb[i][j][:],
                                     start=(i == 0), stop=(i == 3))
                fo = pf.tile([128, 512], F32, name=f"fo{j}_{dt}", tag="fo")
                eng = nc.vector if dt % 2 == 0 else nc.scalar
                eng_copy(nc, eng, fo[:], ps[:], mybir)
                deng = nc.sync if dt % 2 == 0 else nc.scalar
                deng.dma_start(outd[128 * dt:128 * (dt + 1),
                                    512 * j:512 * (j + 1)], fo[:])
    if pm is not None:
        pm.release()
    pB.release()
    pA.release()


def eng_copy(nc, eng, dst, src, mybir):
    """PSUM -> SBUF evacuation copy on the given engine."""
    if eng is nc.scalar:
        nc.scalar.activation(dst, src, mybir.ActivationFunctionType.Copy)
    else:
        eng.tensor_copy(dst, src)


def _rmsnorm_tq(tc, nc, mybir, ADD, cv, eps_t, tq):
    """RMSNorm over R (partition axis of 4 cv tiles) for one token quarter.

    All reduction work on Pool; sqrt on Act; reciprocal on DVE.  cv is
    normalized in place (it becomes c_nrm).
    """
    F32 = mybir.dt.float32
    Sqrt = mybir.ActivationFunctionType.Sqrt
    sl = slice(512 * tq, 512 * (tq + 1))
    with tc.tile_pool(name=f"p2_{tq}", bufs=1) as p2:
        acc = p2.tile([128, 512], F32, name=f"ssq{tq}", tag="ssq")
        tmp = p2.tile([128, 512], F32, name=f"sqt{tq}", tag="sqt")
        nc.gpsimd.tensor_mul(acc[:], cv[0][:, sl], cv[0][:, sl])
        for i in range(1, 4):
            nc.gpsimd.tensor_mul(tmp[:], cv[i][:, sl], cv[i][:, sl])
            nc.gpsimd.tensor_add(acc[:], acc[:], tmp[:])
        nc.gpsimd.partition_all_reduce(acc[:], acc[:], 128, ADD)
        srow = p2.tile([128, 512], F32, name=f"srow{tq}", tag="srow")
        nc.scalar.activation(srow[:], acc[:], Sqrt, bias=eps_t[:], scale=1.0 / R)
        rrow = p2.tile([128, 512], F32, name=f"rrow{tq}", tag="rrow")
        nc.vector.reciprocal(rrow[:], srow[:])
        for i in range(4):
            nc.gpsimd.tensor_mul(cv[i][:, sl], cv[i][:, sl], rrow[:])


# ----------------------------------------------------------------------------
# Host-side input preparation
# ----------------------------------------------------------------------------

_ROPE_PERM = np.empty(DR, dtype=np.int64)
_ROPE_PERM[0::2] = np.arange(32)
_ROPE_PERM[1::2] = np.arange(32, 64)


def _bf16(x):
    import ml_dtypes
    return np.ascontiguousarray(x).astype(ml_dtypes.bfloat16)


def _reorder_headsT(w_shard):
    """[NH*QHD, X] head-major rows -> [X, QROWS] transposed, nope/rope-packed."""
    blocks = []
    for h in range(NH):
        rows = w_shard[h * QHD:(h + 1) * QHD]
        blocks.append(rows[:DN])
    for pair in range(2):
        for h in (2 * pair, 2 * pair + 1):
            rows = w_shard[h * QHD:(h + 1) * QHD]
            blocks.append(rows[DN:][_ROPE_PERM])
    w_re = np.concatenate(blocks, axis=0)  # [768, X]
    return np.ascontiguousarray(w_re.T)


def _build_dmask01():
    """0/1 keep-mask for the 4 diagonal strips; strip d covers chunk c=4j+d."""
    dm = np.ones((128, 4 * 512), dtype=np.float32)
    for d in range(4):
        for m in range(4):
            blk = dm[:, 512 * d + 128 * m: 512 * d + 128 * (m + 1)]
            if m < d:
                blk[:] = 0.0
            elif m == d:
                kk = np.arange(128)[:, None]
                qq = np.arange(128)[None, :]
                blk[:] = np.where(kk > qq, 0.0, 1.0)
    return dm


def _is_causal(mask):
    m = np.asarray(mask).reshape(S, S)
    iu = np.triu_indices(S, 1)
    if not np.all(m[iu] <= -1e8):
        return False
    il = np.tril_indices(S)
    return bool(np.all(m[il] == 0.0))


def _prep_in_maps(inputs):
    hidden = np.asarray(inputs["hidden_states"], dtype=np.float32)
    mask = np.asarray(inputs["attention_mask"], dtype=np.float32)
    position_ids = np.asarray(inputs["position_ids"]).astype(np.int64)
    Wq = np.asarray(inputs["Wq"], dtype=np.float32)
    Wkv_down = np.asarray(inputs["Wkv_down"], dtype=np.float32)
    kv_norm_w = np.asarray(inputs["kv_norm_w"], dtype=np.float32)
    Wkv_up = np.asarray(inputs["Wkv_up"], dtype=np.float32)
    Wkv_v = np.asarray(inputs["Wkv_v"], dtype=np.float32)
    Wo = np.asarray(inputs["Wo"], dtype=np.float32)
    cos = np.asarray(inputs["cos"], dtype=np.float32)
    sin = np.asarray(inputs["sin"], dtype=np.float32)

    causal = _is_causal(mask)

    pos = position_ids.reshape(-1)[:S]
    cos_g = cos[pos]                      # [S, 64]
    sin_g = sin[pos]
    cosT = np.tile(np.ascontiguousarray(cos_g.T)[_ROPE_PERM], (2, 1))
    sinP = np.ascontiguousarray(sin_g.T)[_ROPE_PERM].copy()
    sinP[0::2] = -sinP[0::2]              # row 2i (pairs with d+32): -sin
    sinT = np.tile(sinP, (2, 1))

    Wkv_up_w = Wkv_up * kv_norm_w[None, :]
    Wkv_v_w = Wkv_v * kv_norm_w[None, :]
    wdT = np.ascontiguousarray(Wkv_down.T)  # [HID, R]

    dmask = _build_dmask01() if causal else None
    maskT = None if causal else np.ascontiguousarray(mask.reshape(S, S).T)

    in_maps = []
    for c in range(N_CORES):
        b, g = divmod(c, 4)
        heads = slice(g * NH * QHD, (g + 1) * NH * QHD)
        vh = slice(g * NH * DV, (g + 1) * NH * DV)
        # token-quarter-major hT: [4*HID, 512]
        hT = hidden[b].T                                   # [HID, TOK]
        hTq = np.ascontiguousarray(
            hT.reshape(HID, 4, 512).transpose(1, 0, 2).reshape(4 * HID, 512))
        wqd = np.concatenate(
            [_reorder_headsT(Wq[heads] * np.float32(1.0 / math.sqrt(QHD))),
             wdT], axis=1)                                  # [HID, 1280]
        m = {
            "hTd": _bf16(hTq),
            "wqdd": _bf16(wqd),
            "wud": _bf16(_reorder_headsT(Wkv_up_w[heads])),
            "wvd": _bf16(Wkv_v_w[vh].T),
            "wod": _bf16(Wo[:, vh].T),
            "cosd": _bf16(cosT),
            "sind": _bf16(sinT),
        }
        if causal:
            m["dmd"] = _bf16(dmask)
        else:
            m["maskTd"] = maskT
        in_maps.append(m)
    return causal, in_maps


def _combine(results):
    out = np.zeros((B, S, HID), dtype=np.float32)
    for b in range(B):
        acc = results[4 * b]["out"].astype(np.float64)
        for g in range(1, 4):
            acc = acc + results[4 * b + g]["out"]
        out[b] = acc.T.astype(np.float32)
    return out


def kernel(**inputs):
    from concourse import bass_utils

    causal, in_maps = _prep_in_maps(inputs)
    if causal not in _CACHE:
        _CACHE[causal] = _build(causal)
    nc = _CACHE[causal]

    res = bass_utils.run_bass_kernel_spmd(nc, in_maps, core_ids=list(range(N_CORES)))
    return _combine(res.results)


# revision 13
# speedup vs baseline: 1.9320x; 1.9320x over previous
"""DeepSeek-MLA attention kernel for 8 Trainium2 NeuronCores.

Sharding: tensor-parallel over heads (4 of 16 per core) x data-parallel over
batch (1 of 2 per core-group).  Core c handles batch c//4, heads
[4*(c%4), 4*(c%4)+4).  Each core computes a partial [HID, TOK] output (its
heads' contribution through Wo); the host sums the 4 partials per batch and
transposes back.

Projection paths run in bf16 (PE cost identical to fp32r, half the DMA/SBUF);
attention scores accumulate in f32 PSUM.  Softmax sums ride the Pool engine
(tensor_add tree + partition_all_reduce) instead of PE ones-matmuls, and the
causal mask is a 0/1 multiply on the exp output (Pool) rather than f32 adds
on PSUM.  Weights are loaded once; hidden states stream in token quarters
with fully-contiguous DMA.  RoPE uses a pair-interleaved row permutation so
rotate_half is a 32-lane stream_shuffle on the vector engine.
"""

import math
import sys

import numpy as np

for _p in ("/opt/trn_rl_repo", "/root/.axon_site/_ro/trn_rl_repo"):
    if _p not in sys.path:
        sys.path.append(_p)

# Problem dims (hardcoded per contract)
B, S, HID = 2, 2048, 2048
H, DN, DR, DV, R = 16, 128, 64, 128, 512
QHD = DN + DR  # 192
EPS = 1e-5
N_CORES = 8
NH = 4                 # heads per core
TOK = S                # tokens per core (one batch)
QROWS = NH * QHD       # 768 = 4*128 nope + 2*128 packed rope
WCOLS = QROWS + R      # 1280 fused wq|wd columns
NEG = -30000.0

_CACHE = {}

_SHUF_MASK = []
for _i in range(16):
    _SHUF_MASK += [2 * _i + 1, 2 * _i]


# ----------------------------------------------------------------------------
# Device program
# ----------------------------------------------------------------------------

def _build(causal: bool):
    import concourse.mybir as mybir
    import concourse.tile as tile
    from concourse import bacc

    F32 = mybir.dt.float32
    BF16 = mybir.dt.bfloat16

    nc = bacc.Bacc("TRN2", target_bir_lowering=False, debug=False,
                   enable_asserts=False, num_devices=N_CORES)

    # hT stored token-quarter-major: rows 2048*tq + 128*c .. +128 hold HID
    # chunk c for token quarter tq; each [128, 512] chunk is contiguous.
    hTd = nc.dram_tensor("hTd", [4 * HID, 512], BF16, kind="ExternalInput").ap()
    wqdd = nc.dram_tensor("wqdd", [HID, WCOLS], BF16, kind="ExternalInput").ap()
    wud = nc.dram_tensor("wud", [R, QROWS], BF16, kind="ExternalInput").ap()
    wvd = nc.dram_tensor("wvd", [R, NH * DV], BF16, kind="ExternalInput").ap()
    wod = nc.dram_tensor("wod", [NH * DV, HID], BF16, kind="ExternalInput").ap()
    cosd = nc.dram_tensor("cosd", [128, TOK], BF16, kind="ExternalInput").ap()
    sind = nc.dram_tensor("sind", [128, TOK], BF16, kind="ExternalInput").ap()
    if causal:
        dmd = nc.dram_tensor("dmd", [128, 4 * 512], BF16, kind="ExternalInput").ap()
    else:
        maskTd = nc.dram_tensor("maskTd", [S, S], F32, kind="ExternalInput").ap()
    outd = nc.dram_tensor("out", [HID, TOK], F32, kind="ExternalOutput").ap()

    with tile.TileContext(nc) as tc:
        with nc.allow_low_precision(reason="bf16/f32r mixed-precision kernel"):
            _emit(tc, nc, mybir, causal,
                  hTd, wqdd, wud, wvd, wod, cosd, sind,
                  dmd if causal else maskTd, outd)
    nc.compile()
    return nc


def _emit(tc, nc, mybir, causal,
          hTd, wqdd, wud, wvd, wod, cosd, sind, maskd, outd):
    from concourse import bass_isa

    F32 = mybir.dt.float32
    BF16 = mybir.dt.bfloat16
    Exp = mybir.ActivationFunctionType.Exp
    Sqrt = mybir.ActivationFunctionType.Sqrt
    ADD = bass_isa.ReduceOp.add

    # --- pool A: tiles visible during P1 that live past it ---
    pA = tc.alloc_tile_pool(name="A", bufs=1)
    dm01 = pA.tile([128, 4 * 512], BF16, name="dm01", tag="dm01") if causal else None
    eps_t = pA.tile([128, 1], F32, name="eps_t", tag="eps_t")
    nc.vector.memset(eps_t[:], EPS)
    qn = [pA.tile([128, TOK], BF16, name=f"qn{h}", tag=f"qn{h}") for h in range(4)]
    qr = [pA.tile([128, TOK], BF16, name=f"qr{p}", tag=f"qr{p}") for p in range(2)]
    cos_t = pA.tile([128, TOK], BF16, name="cos_t", tag="cos_t")
    sin_t = pA.tile([128, TOK], BF16, name="sin_t", tag="sin_t")
    cv = [pA.tile([128, TOK], BF16, name=f"cv{i}", tag=f"cv{i}") for i in range(4)]
    wu_ch = [pA.tile([128, QROWS], BF16, name=f"wu{i}", tag=f"wu{i}")
             for i in range(4)]
    wv_ch = [pA.tile([128, NH * DV], BF16, name=f"wv{i}", tag=f"wv{i}")
             for i in range(4)]
    kr_raw = [pA.tile([128, TOK], BF16, name=f"kr_raw{p}", tag=f"kr_raw{p}")
              for p in range(2)]
    qr_raw = [pA.tile([128, TOK], BF16, name=f"qr_raw{p}", tag=f"qr_raw{p}")
              for p in range(2)]

    # ------------------------------------------------------------------
    # P1: q / c_kv projections, streamed in token quarters.
    #     M-tile columns in wqd: 0-511 qn(4), 512-767 qr(2), 768-1279 cv(4)
    # ------------------------------------------------------------------
    pP1 = tc.alloc_tile_pool(name="P1", bufs=1)
    wqd_ch = [pP1.tile([128, WCOLS], BF16, name=f"wqd{c}", tag=f"wqd{c}")
              for c in range(16)]

    # m-tile order: cv first (so RMSNorm can overlap), then qn, then qr
    m_tiles = ([(768 + 128 * i, cv[i]) for i in range(4)]
               + [(128 * h, qn[h]) for h in range(4)]
               + [(512 + 128 * p, qr_raw[p]) for p in range(2)])

    with tc.tile_pool(name="ht", bufs=20) as ph, \
         tc.tile_pool(name="p1ps", bufs=4, space="PSUM") as pps:
        # critical-path loads first: wqd chunk + tq0 hT chunk per c, both
        # queues fed; aux loads (cos/sin/mask/wu/wv) follow.
        ht0 = []
        for c in range(16):
            e1, e2 = (nc.sync, nc.scalar) if c % 2 == 0 else (nc.scalar, nc.sync)
            e1.dma_start(wqd_ch[c][:], wqdd[128 * c:128 * (c + 1), :])
            t = ph.tile([128, 512], BF16, name=f"ht0_{c}", tag="ht")
            e2.dma_start(t[:], hTd[128 * c:128 * (c + 1), :])
            ht0.append(t)
        nc.sync.dma_start(cos_t[:], cosd)
        nc.sync.dma_start(sin_t[:], sind)
        if causal:
            nc.sync.dma_start(dm01[:], maskd)
        for i in range(4):
            nc.sync.dma_start(wu_ch[i][:], wud[128 * i:128 * (i + 1), :])
            nc.sync.dma_start(wv_ch[i][:], wvd[128 * i:128 * (i + 1), :])

        for tq in range(4):
            if tq == 0:
                ht = ht0
            else:
                ht = []
                for c in range(16):
                    t = ph.tile([128, 512], BF16, name=f"ht{tq}_{c}", tag="ht")
                    eng = nc.sync if c % 2 == 0 else nc.scalar
                    eng.dma_start(t[:], hTd[2048 * tq + 128 * c:
                                            2048 * tq + 128 * (c + 1), :])
                    ht.append(t)
            for mi, (mcol, target) in enumerate(m_tiles):
                ps = pps.tile([128, 512], F32, name=f"p1ps{tq}_{mi}", tag="p1ps")
                for c in range(16):
                    nc.tensor.matmul(ps[:], wqd_ch[c][:, mcol:mcol + 128],
                                     ht[c][:], start=(c == 0), stop=(c == 15))
                nc.vector.tensor_copy(target[:, 512 * tq:512 * (tq + 1)], ps[:])
                if mi == 3:
                    # P2: RMSNorm for this quarter (Pool/Act/DVE, no PE) —
                    # emitted early so its serial tail hides under qn/qr passes
                    _rmsnorm_tq(tc, nc, mybir, ADD, cv, eps_t, tq)
    pP1.release()

    # --- pool B: P3 outputs + output-projection tiles (fit where P1 was) ---
    pB = tc.alloc_tile_pool(name="Bpool", bufs=1)
    kn = [pB.tile([128, TOK], BF16, name=f"kn{h}", tag=f"kn{h}") for h in range(4)]
    kr = [pB.tile([128, TOK], BF16, name=f"kr{p}", tag=f"kr{p}") for p in range(2)]
    v_sb = [pB.tile([128, NH * DV], BF16, name=f"v{t}", tag=f"v{t}")
            for t in range(16)]
    o_sb = [[pB.tile([128, 512], BF16, name=f"o{h}_{j}", tag=f"o{h}_{j}")
             for j in range(4)] for h in range(4)]
    wo_ch = [pB.tile([128, HID], BF16, name=f"wo{i}", tag=f"wo{i}")
             for i in range(4)]

    # ------------------------------------------------------------------
    # P3a: k up-projection (+ k RoPE);  P3b: v up-projection
    # ------------------------------------------------------------------
    k_out = [(kr_raw[0], 512), (kr_raw[1], 640),
             (kn[0], 0), (kn[1], 128), (kn[2], 256), (kn[3], 384)]
    with tc.tile_pool(name="p3ps", bufs=4, space="PSUM") as ppk:
        for mi, (target, coff) in enumerate(k_out):
            for s in range(4):
                ps = ppk.tile([128, 512], F32, name=f"p3k{mi}_{s}", tag="p3k")
                for i in range(4):
                    nc.tensor.matmul(ps[:], wu_ch[i][:, coff:coff + 128],
                                     cv[i][:, 512 * s:512 * (s + 1)],
                                     start=(i == 0), stop=(i == 3))
                if (mi * 4 + s) % 2 == 0:
                    nc.scalar.activation(target[:, 512 * s:512 * (s + 1)], ps[:],
                                         mybir.ActivationFunctionType.Copy)
                else:
                    nc.vector.tensor_copy(target[:, 512 * s:512 * (s + 1)], ps[:])
    with tc.tile_pool(name="krope", bufs=2) as pkt:
        for p in range(2):
            tmp = pkt.tile([128, TOK], BF16, name=f"kropetmp{p}", tag="kropetmp")
            nc.vector.stream_shuffle(tmp[:], kr_raw[p][:], _SHUF_MASK)
            nc.vector.tensor_mul(tmp[:], tmp[:], sin_t[:])
            nc.vector.tensor_mul(kr_raw[p][:], kr_raw[p][:], cos_t[:])
            nc.vector.tensor_add(kr[p][:], kr_raw[p][:], tmp[:])
    with tc.tile_pool(name="p3vps", bufs=4, space="PSUM") as ppv:
        for tt in range(16):
            ps = ppv.tile([128, NH * DV], F32, name=f"p3v{tt}", tag="p3v")
            for i in range(4):
                nc.tensor.matmul(ps[:], cv[i][:, 128 * tt:128 * (tt + 1)],
                                 wv_ch[i][:], start=(i == 0), stop=(i == 3))
            if tt % 2 == 1:
                nc.scalar.activation(v_sb[tt][:], ps[:],
                                     mybir.ActivationFunctionType.Copy)
            else:
                nc.vector.tensor_copy(v_sb[tt][:], ps[:])

    # RoPE on q (DVE; overlaps P3b PE work)
    with tc.tile_pool(name="qrope", bufs=2) as pr:
        for p in range(2):
            tmp = pr.tile([128, TOK], BF16, name=f"qropetmp{p}", tag="qropetmp")
            nc.vector.stream_shuffle(tmp[:], qr_raw[p][:], _SHUF_MASK)
            nc.vector.tensor_mul(tmp[:], tmp[:], sin_t[:])
            nc.vector.tensor_mul(qr_raw[p][:], qr_raw[p][:], cos_t[:])
            nc.vector.tensor_add(qr[p][:], qr_raw[p][:], tmp[:])

    # wo loads (Act queue is about to go exp-only; issue on SP)
    for i in range(4):
        nc.sync.dma_start(wo_ch[i][:], wod[128 * i:128 * (i + 1), :])

    # ------------------------------------------------------------------
    # P4: attention per head; transposed scores sT[k, q].
    #     Sum of exp rides Pool (add tree + partition_all_reduce).
    # ------------------------------------------------------------------
    pm = None if causal else tc.alloc_tile_pool(name="mload", bufs=4)
    with tc.tile_pool(name="exp", bufs=6) as pe_, \
         tc.tile_pool(name="acc", bufs=2) as pa_, \
         tc.tile_pool(name="norm", bufs=4) as pn, \
         tc.tile_pool(name="fout", bufs=4) as pf, \
         tc.tile_pool(name="qkps", bufs=3, space="PSUM") as pqk, \
         tc.tile_pool(name="pvps", bufs=3, space="PSUM") as ppv4, \
         tc.tile_pool(name="p5ps", bufs=2, space="PSUM") as pps5:
        for j in range(4):
            nch = 4 * (j + 1) if causal else 16
            for h in range(4):
                p = h // 2
                rs0 = 64 * (h % 2)
                pv_ps = ppv4.tile([128, 512], F32, name=f"pv{h}_{j}", tag="pv")
                acc = pa_.tile([128, 512], F32, name=f"acc{h}_{j}", tag="acc")
                for ci in range(nch):
                    c = ci
                    qk_ps = pqk.tile([128, 512], F32, name=f"qk{h}_{j}_{c}", tag="qk")
                    nc.tensor.matmul(qk_ps[:],
                                     kn[h][:, 128 * c:128 * (c + 1)],
                                     qn[h][:, 512 * j:512 * (j + 1)],
                                     start=True, stop=False)
                    nc.tensor.matmul(qk_ps[:],
                                     kr[p][rs0:rs0 + 64, 128 * c:128 * (c + 1)],
                                     qr[p][rs0:rs0 + 64, 512 * j:512 * (j + 1)],
                                     start=False, stop=True)
                    if not causal:
                        mt = pm.tile([128, 512], F32, name=f"mt{h}{j}{c}", tag="mt")
                        eng = nc.sync if ci % 2 == 0 else nc.scalar
                        eng.dma_start(mt[:], maskd[128 * c:128 * (c + 1),
                                                   512 * j:512 * (j + 1)])
                        nc.vector.tensor_add(qk_ps[:], qk_ps[:], mt[:])
                    e = pe_.tile([128, 512], BF16, name=f"e{h}{j}{c}", tag="e")
                    nc.scalar.activation(e[:], qk_ps[:], Exp)
                    if causal:
                        d = c - 4 * j
                        if d >= 0:
                            nc.gpsimd.tensor_mul(e[:], e[:],
                                                 dm01[:, 512 * d:512 * (d + 1)])
                    if ci == 0:
                        nc.gpsimd.tensor_copy(acc[:], e[:])
                    else:
                        nc.gpsimd.tensor_add(acc[:], acc[:], e[:])
                    nc.tensor.matmul(pv_ps[:],
                                     v_sb[c][:, 128 * h:128 * (h + 1)],
                                     e[:],
                                     start=(ci == 0), stop=(ci == nch - 1))
                nc.gpsimd.partition_all_reduce(acc[:], acc[:], 128, ADD)
                rr = pn.tile([128, 512], F32, name=f"rr{h}{j}", tag="rr")
                nc.vector.reciprocal(rr[:], acc[:])
                nc.vector.tensor_mul(o_sb[h][j][:], pv_ps[:], rr[:])
            # P5 for this q block: fills exp-wait bubbles, shrinks the drain
            for dt in range(16):
                ps = pps5.tile([128, 512], F32, name=f"p5_{j}_{dt}", tag="p5")
                for i in range(4):
                    nc.tensor.matmul(ps[:],
                                     wo_ch[i][:, 128 * dt:128 * (dt + 1)],
                                     o_sb[i][j][:],
                                     start=(i == 0), stop=(i == 3))
                fo = pf.tile([128, 512], F32, name=f"fo{j}_{dt}", tag="fo")
                # DVE-only evac + SP-only DMA: Act must stay exp-only here,
                # or the Copy<->Exp act-table reloads stall the exp chain
                nc.vector.tensor_copy(fo[:], ps[:])
                nc.sync.dma_start(outd[128 * dt:128 * (dt + 1),
                                       512 * j:512 * (j + 1)], fo[:])
    if pm is not None:
        pm.release()
    pB.release()
    pA.release()


def eng_copy(nc, eng, dst, src, mybir):
    """PSUM -> SBUF evacuation copy on the given engine."""
    if eng is nc.scalar:
        nc.scalar.activation(dst, src, mybir.ActivationFunctionType.Copy)
    else:
        eng.tensor_copy(dst, src)


def _rmsnorm_tq(tc, nc, mybir, ADD, cv, eps_t, tq):
    """RMSNorm over R (partition axis of 4 cv tiles) for one token quarter.

    All reduction work on Pool; sqrt on Act; reciprocal on DVE.  cv is
    normalized in place (it becomes c_nrm).
    """
    F32 = mybir.dt.float32
    Sqrt = mybir.ActivationFunctionType.Sqrt
    sl = slice(512 * tq, 512 * (tq + 1))
    with tc.tile_pool(name=f"p2_{tq}", bufs=1) as p2:
        acc = p2.tile([128, 512], F32, name=f"ssq{tq}", tag="ssq")
        tmp = p2.tile([128, 512], F32, name=f"sqt{tq}", tag="sqt")
        nc.gpsimd.tensor_mul(acc[:], cv[0][:, sl], cv[0][:, sl])
        for i in range(1, 4):
            nc.gpsimd.tensor_mul(tmp[:], cv[i][:, sl], cv[i][:, sl])
            nc.gpsimd.tensor_add(acc[:], acc[:], tmp[:])
        nc.gpsimd.partition_all_reduce(acc[:], acc[:], 128, ADD)
        srow = p2.tile([128, 512], F32, name=f"srow{tq}", tag="srow")
        nc.scalar.activation(srow[:], acc[:], Sqrt, bias=eps_t[:], scale=1.0 / R)
        rrow = p2.tile([128, 512], F32, name=f"rrow{tq}", tag="rrow")
        nc.vector.reciprocal(rrow[:], srow[:])
        for i in range(4):
            nc.gpsimd.tensor_mul(cv[i][:, sl], cv[i][:, sl], rrow[:])


# ----------------------------------------------------------------------------
# Host-side input preparation
# ----------------------------------------------------------------------------

_ROPE_PERM = np.empty(DR, dtype=np.int64)
_ROPE_PERM[0::2] = np.arange(32)
_ROPE_PERM[1::2] = np.arange(32, 64)


def _bf16(x):
    import ml_dtypes
    return np.ascontiguousarray(x).astype(ml_dtypes.bfloat16)


def _reorder_headsT(w_shard):
    """[NH*QHD, X] head-major rows -> [X, QROWS] transposed, nope/rope-packed."""
    blocks = []
    for h in range(NH):
        rows = w_shard[h * QHD:(h + 1) * QHD]
        blocks.append(rows[:DN])
    for pair in range(2):
        for h in (2 * pair, 2 * pair + 1):
            rows = w_shard[h * QHD:(h + 1) * QHD]
            blocks.append(rows[DN:][_ROPE_PERM])
    w_re = np.concatenate(blocks, axis=0)  # [768, X]
    return np.ascontiguousarray(w_re.T)


def _build_dmask01():
    """0/1 keep-mask for the 4 diagonal strips; strip d covers chunk c=4j+d."""
    dm = np.ones((128, 4 * 512), dtype=np.float32)
    for d in range(4):
        for m in range(4):
            blk = dm[:, 512 * d + 128 * m: 512 * d + 128 * (m + 1)]
            if m < d:
                blk[:] = 0.0
            elif m == d:
                kk = np.arange(128)[:, None]
                qq = np.arange(128)[None, :]
                blk[:] = np.where(kk > qq, 0.0, 1.0)
    return dm


def _is_causal(mask):
    m = np.asarray(mask).reshape(S, S)
    iu = np.triu_indices(S, 1)
    if not np.all(m[iu] <= -1e8):
        return False
    il = np.tril_indices(S)
    return bool(np.all(m[il] == 0.0))


def _prep_in_maps(inputs):
    hidden = np.asarray(inputs["hidden_states"], dtype=np.float32)
    mask = np.asarray(inputs["attention_mask"], dtype=np.float32)
    position_ids = np.asarray(inputs["position_ids"]).astype(np.int64)
    Wq = np.asarray(inputs["Wq"], dtype=np.float32)
    Wkv_down = np.asarray(inputs["Wkv_down"], dtype=np.float32)
    kv_norm_w = np.asarray(inputs["kv_norm_w"], dtype=np.float32)
    Wkv_up = np.asarray(inputs["Wkv_up"], dtype=np.float32)
    Wkv_v = np.asarray(inputs["Wkv_v"], dtype=np.float32)
    Wo = np.asarray(inputs["Wo"], dtype=np.float32)
    cos = np.asarray(inputs["cos"], dtype=np.float32)
    sin = np.asarray(inputs["sin"], dtype=np.float32)

    causal = _is_causal(mask)

    pos = position_ids.reshape(-1)[:S]
    cos_g = cos[pos]                      # [S, 64]
    sin_g = sin[pos]
    cosT = np.tile(np.ascontiguousarray(cos_g.T)[_ROPE_PERM], (2, 1))
    sinP = np.ascontiguousarray(sin_g.T)[_ROPE_PERM].copy()
    sinP[0::2] = -sinP[0::2]              # row 2i (pairs with d+32): -sin
    sinT = np.tile(sinP, (2, 1))

    Wkv_up_w = Wkv_up * kv_norm_w[None, :]
    Wkv_v_w = Wkv_v * kv_norm_w[None, :]
    wdT = np.ascontiguousarray(Wkv_down.T)  # [HID, R]

    dmask = _build_dmask01() if causal else None
    maskT = None if causal else np.ascontiguousarray(mask.reshape(S, S).T)

    in_maps = []
    for c in range(N_CORES):
        b, g = divmod(c, 4)
        heads = slice(g * NH * QHD, (g + 1) * NH * QHD)
        vh = slice(g * NH * DV, (g + 1) * NH * DV)
        # token-quarter-major hT: [4*HID, 512]
        hT = hidden[b].T                                   # [HID, TOK]
        hTq = np.ascontiguousarray(
            hT.reshape(HID, 4, 512).transpose(1, 0, 2).reshape(4 * HID, 512))
        wqd = np.concatenate(
            [_reorder_headsT(Wq[heads] * np.float32(1.0 / math.sqrt(QHD))),
             wdT], axis=1)                                  # [HID, 1280]
        m = {
            "hTd": _bf16(hTq),
            "wqdd": _bf16(wqd),
            "wud": _bf16(_reorder_headsT(Wkv_up_w[heads])),
            "wvd": _bf16(Wkv_v_w[vh].T),
            "wod": _bf16(Wo[:, vh].T),
            "cosd": _bf16(cosT),
            "sind": _bf16(sinT),
        }
        if causal:
            m["dmd"] = _bf16(dmask)
        else:
            m["maskTd"] = maskT
        in_maps.append(m)
    return causal, in_maps


def _combine(results):
    out = np.zeros((B, S, HID), dtype=np.float32)
    for b in range(B):
        acc = results[4 * b]["out"].astype(np.float64)
        for g in range(1, 4):
            acc = acc + results[4 * b + g]["out"]
        out[b] = acc.T.astype(np.float32)
    return out


def kernel(**inputs):
    from concourse import bass_utils

    causal, in_maps = _prep_in_maps(inputs)
    if causal not in _CACHE:
        _CACHE[causal] = _build(causal)
    nc = _CACHE[causal]

    res = bass_utils.run_bass_kernel_spmd(nc, in_maps, core_ids=list(range(N_CORES)))
    return _combine(res.results)
